# revision 35
# baseline (speedup 1.0000x reference)
"""Trainium2 Bass kernel for MixtureOfSoftmaxes (v3).

Module: RMSNorm -> gate MLP (silu, softmax over K experts) -> big GEMM
x @ expert_w (H=1024 -> K*V=128000), softmax over V per expert, mix with
gate weights, log.

Sharding: tensor-parallel over vocab. Core c computes, for all K=4
experts, the vocab window [c*4000, (c+1)*4000) (padded to 4096/expert).
Per token group (2 blocks of 128 tokens) each core AllGathers its local
softmax partial sums (4 KB) and reduces them on DVE; the mix then uses
a = softmax(gate) / Z.

v3 structure:
 - Host pre-transposes x to [H, T] (bf16 + fp8) and pre-blocks W into
   contiguous 256 KB (col-chunk, hs) tiles for full-rate DMA.
 - RMS rinv via ones-matmul column reduction + exp(-0.5*ln(v)); squares
   taken from the fp8 copy (arrives first). rinv folds into the big
   GEMM's exp as its per-partition scale; the gate path normalizes xT
   in place.
 - Big GEMM: 4 token groups x 16 col-chunks of 1024; psum tiles
   [128,1024]; wide exp+row-sum ACTIVATE per (t-block, chunk); P kept
   in SBUF bf16. First col-chunk's matmuls are emitted ahead of the
   norm matmuls so the PE starts at ~+4us.
 - Collective bounce DMAs ride the GpSimd queue so the Sync queue's W
   stream never blocks on an AllGather.
 - Groups 0-2 mix on DVE (hidden under the next group's GEMM); the
   last group mixes on the then-idle PE via accumulating diag(a_k)@P_k
   matmuls, with Ln reading straight from PSUM.
"""

import sys

sys.path.insert(0, "/opt/trn_rl_repo")

import numpy as np
import ml_dtypes

import concourse.bass as bass
import concourse.bacc as bacc
import concourse.mybir as mybir
import concourse.tile as tile
from concourse.bass_utils import run_bass_kernel_spmd
from concourse.masks import make_identity

AFT = mybir.ActivationFunctionType
F32 = mybir.dt.float32
BF16 = mybir.dt.bfloat16
FP8 = mybir.dt.float8e4
FP8NP = ml_dtypes.float8_e4m3
WSCALE = 16.0

B, S, H, K, V = 2, 512, 1024, 4, 32000
T = B * S              # 1024 tokens
NC = 8                 # cores
VSH = V // NC          # 4000 vocab cols per core per expert
VP = 4096              # padded per-expert width
C = K * VP             # 16384 GEMM cols per core
D = H // 2             # 512 gate hidden
EPS_RMS = 1e-5
EPS_LOG = 1e-10
TB = T // 128          # 8 token blocks
HB = H // 128          # 8 contraction blocks
NG = C // 1024         # 16 col chunks of 1024
GROUPS = [(0, 2), (2, 2), (4, 2), (6, 2)]  # (start t-block, count)


_ONE_SET = "natural_log_exp_and_others"
_orig_gat = None


def _single_set_tables(arch):
    """Keep id->set alignment but strip our functions from every set except
    the combined exp+ln one, so the table-load pass never alternates sets."""
    d = _orig_gat(arch)
    if _ONE_SET not in d:
        return d
    combined = d[_ONE_SET]
    return {name: (fns if name == _ONE_SET else fns - combined)
            for name, fns in d.items()}


def build_kernel():
    global _orig_gat
    if _orig_gat is None:
        _orig_gat = bacc.get_activation_tables
        bacc.get_activation_tables = _single_set_tables
    nc = bacc.Bacc("TRN2", target_bir_lowering=False, debug=False, num_devices=NC)
    xT_d = nc.dram_tensor("xt", [H, T], BF16, kind="ExternalInput")
    xn_d = nc.dram_tensor("xn", [TB, 128, H], FP8, kind="ExternalInput")
    x8_d = nc.dram_tensor("x8", [H, T], FP8, kind="ExternalInput")
    # W pre-blocked on host: [gG, hs, p, j, c] so each (gG, hs) chunk is
    # one contiguous 256 KB read.
    w_d = nc.dram_tensor("w", [NG, HB // 2, 128, 2, 1024], FP8,
                         kind="ExternalInput")
    wd_d = nc.dram_tensor("wd", [H, D], BF16, kind="ExternalInput")
    wu_d = nc.dram_tensor("wu", [D, K], BF16, kind="ExternalInput")
    bd_d = nc.dram_tensor("bd", [D, 1], F32, kind="ExternalInput")
    nbd_d = nc.dram_tensor("nbd", [D, 1], F32, kind="ExternalInput")
    bu_d = nc.dram_tensor("bu", [128, K], F32, kind="ExternalInput")
    o_d = nc.dram_tensor("o", [TB, 128, VSH], BF16, kind="ExternalOutput")

    xT_ap = xT_d.rearrange("(hb p) t -> p hb t", p=128)
    x8_ap = x8_d.rearrange("(hb p) t -> p hb t", p=128)
    wd_ap = wd_d.rearrange("(hb p) d -> p hb d", p=128)
    wu_ap = wu_d.rearrange("(db p) k -> p db k", p=128)
    bd_ap = bd_d.rearrange("(db p) o -> p db o", p=128)
    nbd_ap = nbd_d.rearrange("(db p) o -> p db o", p=128)

    with tile.TileContext(nc) as tc:
        with tc.tile_pool(name="persist", bufs=1) as pers, \
             tc.tile_pool(name="ps", bufs=1, space="PSUM") as ps, \
             tc.tile_pool(name="ccdr", bufs=1, space="DRAM") as ccdr:
            # ---- persistent small tiles ----
            identB = pers.tile([128, 128], BF16)
            make_identity(nc, identB[:])
            ones = pers.tile([128, 128], BF16)
            nc.gpsimd.memset(ones[:], 1.0)
            eps_rms = pers.tile([128, 1], F32)
            nc.gpsimd.memset(eps_rms[:], EPS_RMS)
            eps_log = pers.tile([128, 1], F32)
            nc.gpsimd.memset(eps_log[:], EPS_LOG)
            xT = pers.tile([128, HB, T], BF16)    # 16 KB/part (normalized in place)
            x8 = pers.tile([128, HB, T], FP8)     # 8 KB/part (raw x, fp8)
            scl = pers.tile([128, TB], F32)       # rinv/WSCALE per t-block
            gw = pers.tile([128, TB, K], F32)     # gate softmax weights
            wd_sb = pers.tile([128, HB, D], BF16)   # 8 KB/part
            wu_sb = pers.tile([128, D // 128, K], BF16)
            bd_sb = pers.tile([128, D // 128, 1], F32)
            nbd_sb = pers.tile([128, D // 128, 1], F32)
            buT_sb = pers.tile([128, K], F32)
            gT = pers.tile([128, D // 128, T], BF16)  # 8 KB/part

            # ---- input DMAs (x8 chunked first: feeds squares + GEMM) ----
            for hb in range(HB):
                nc.sync.dma_start(x8[:, hb, :], x8_ap[:, hb, :])
            xn = pers.tile([128, TB, H], FP8)     # 8 KB/part (fast scl path)
            # scalar-queue (qActDynamicHW) so xn lands in parallel with x8
            for tb in range(TB):
                nc.scalar.dma_start(xn[:, tb, :], xn_d[tb])
            for hb in range(HB):
                nc.sync.dma_start(xT[:, hb, :], xT_ap[:, hb, :])
            nc.sync.dma_start(wd_sb[:], wd_ap)
            nc.sync.dma_start(wu_sb[:], wu_ap)
            nc.sync.dma_start(bd_sb[:], bd_ap)
            nc.sync.dma_start(nbd_sb[:], nbd_ap)
            nc.sync.dma_start(buT_sb[:], bu_d[:])
            # warmup AllGather: pays the collective first-call staging cost
            # off the critical path (overlaps the norm + first GEMM chunks)
            wup = pers.tile([128, 8], F32)
            nc.gpsimd.memset(wup[:], 0.0)
            bi_w = ccdr.tile([128, 8], F32, tag="bi", bufs=2, name="bi_w")
            bo_w = ccdr.tile([NC * 128, 8], F32, tag="bo", bufs=2, name="bo_w")
            nc.gpsimd.dma_start(bi_w[:], wup[:])
            nc.gpsimd.collective_compute(
                "AllGather", mybir.AluOpType.bypass,
                replica_groups=[list(range(NC))],
                ins=[bi_w[:]], outs=[bo_w[:]],
            )

            with tc.tile_pool(name="main", bufs=1) as mp:

                def emit_exp_chunk(g, ts, cnt, gG, pts, ssum, pp):
                    for t2 in range(cnt):
                        t = ts + t2
                        kk, cc = gG // (NG // K), gG % (NG // K)
                        nc.scalar.activation(
                            pts[t2][:, gG * 1024 : (gG + 1) * 1024],
                            pp[t2][:], AFT.Exp,
                            bias=0.0, scale=scl[:, t : t + 1],
                            accum_out=ssum[:, t2, kk, cc : cc + 1],
                        )

                def emit_gemm_chunk(g, ts, cnt, gG, pts, ssum, emit_exp=True):
                    """One 1024-col chunk of a token group's GEMM."""
                    pp = []
                    for t2 in range(cnt):
                        pp.append(ps.tile([128, 1024], F32, tag="mm", bufs=4,
                                          name=f"mm{g}_{gG}_{t2}"))
                    for hs in range(HB // 2):
                        wt = mp.tile([128, 2, 1024], FP8, tag="wt", bufs=6,
                                     name=f"wt{g}_{gG}_{hs}")
                        nc.sync.dma_start(wt[:], w_d[gG, hs])
                        for t2 in range(cnt):
                            t = ts + t2
                            for ch in range(2):
                                nc.tensor.matmul(
                                    pp[t2][:, ch * 512 : (ch + 1) * 512],
                                    lhsT=x8[:, 2 * hs : 2 * hs + 2,
                                            t * 128 : (t + 1) * 128],
                                    rhs=wt[:, :, ch * 512 : (ch + 1) * 512],
                                    start=(hs == 0), stop=(hs == HB // 2 - 1),
                                    perf_mode=mybir.MatmulPerfMode.DoubleRow,
                                )
                    if emit_exp:
                        emit_exp_chunk(g, ts, cnt, gG, pts, ssum, pp)
                    return pp

                def alloc_group(g, cnt):
                    ssum = mp.tile([128, cnt, K, NG // K], F32, tag="ssum",
                                   bufs=2, name=f"ssum{g}")
                    pts = [mp.tile([128, C], BF16, tag="P", bufs=4,
                                   name=f"P{g}_{t2}") for t2 in range(cnt)]
                    return ssum, pts

                def emit_gate():
                    for d in range(D // 128):
                        for half in range(2):
                            sl = slice(half * 512, (half + 1) * 512)
                            pgt = ps.tile([128, 1024], F32, tag="mm",
                                          name=f"pg{d}_{half}", bufs=4)
                            pg = pgt[:, :512]
                            for hb in range(HB):
                                nc.tensor.matmul(
                                    pg,
                                    lhsT=wd_sb[:, hb, d * 128 : (d + 1) * 128],
                                    rhs=xT[:, hb, sl],
                                    start=(hb == 0), stop=(hb == HB - 1),
                                )
                            lin = mp.tile([128, 512], BF16, tag="glin", bufs=1,
                                          name=f"lin{d}_{half}")
                            nc.scalar.activation(lin[:], pg, AFT.Identity,
                                                 bias=bd_sb[:, d, :], scale=1.0)
                            ex = pgt[:, 512:]
                            nc.scalar.activation(ex, pg, AFT.Exp,
                                                 bias=nbd_sb[:, d, :], scale=-1.0)
                            nc.vector.tensor_scalar_add(ex, ex, 1.0)
                            nc.vector.reciprocal_approx_fast(ex, ex)
                            nc.vector.tensor_mul(gT[:, d, sl], lin[:], ex)
                    glt = mp.tile([128, TB, K], F32, tag="glt", bufs=1)
                    for tb in range(TB):
                        pgl = ps.tile([128, 1024], F32, tag="mm",
                                      name=f"pgl{tb}", bufs=4)
                        for d in range(D // 128):
                            nc.tensor.matmul(
                                pgl[:, :4],
                                lhsT=gT[:, d, tb * 128 : (tb + 1) * 128],
                                rhs=wu_sb[:, d, :],
                                start=(d == 0), stop=(d == D // 128 - 1),
                            )
                        nc.vector.tensor_add(glt[:, tb, :], pgl[:, :4],
                                             buT_sb[:])
                    negm = mp.tile([128, TB], F32, tag="negm", bufs=1)
                    esum = mp.tile([128, TB], F32, tag="esum", bufs=1)
                    for tb in range(TB):
                        nc.vector.tensor_reduce(
                            negm[:, tb : tb + 1], glt[:, tb, :],
                            axis=mybir.AxisListType.X, op=mybir.AluOpType.max,
                            negate=True,
                        )
                        nc.scalar.activation(gw[:, tb, :], glt[:, tb, :],
                                             AFT.Exp,
                                             bias=negm[:, tb : tb + 1],
                                             scale=1.0,
                                             accum_out=esum[:, tb : tb + 1])
                    rsum = mp.tile([128, TB], F32, tag="rsum", bufs=1)
                    nc.vector.reciprocal(rsum[:], esum[:])
                    for tb in range(TB):
                        nc.vector.tensor_scalar_mul(gw[:, tb, :], gw[:, tb, :],
                                                    rsum[:, tb : tb + 1])

                def emit_group_sync(g, ts, cnt, ssum):
                    s_g = mp.tile([128, cnt, K], F32, tag="s_g", bufs=2,
                                  name=f"s_g{g}")
                    nc.vector.tensor_reduce(
                        s_g[:], ssum[:],
                        axis=mybir.AxisListType.X, op=mybir.AluOpType.add,
                    )
                    bi = ccdr.tile([128, cnt * K], F32, tag="bi", bufs=2,
                                   name=f"bi{g}")
                    bo = ccdr.tile([NC * 128, cnt * K], F32, tag="bo", bufs=2,
                                   name=f"bo{g}")
                    nc.gpsimd.dma_start(bi[:],
                                        s_g[:].rearrange("p t k -> p (t k)"))
                    nc.gpsimd.collective_compute(
                        "AllGather", mybir.AluOpType.bypass,
                        replica_groups=[list(range(NC))],
                        ins=[bi[:]], outs=[bo[:]],
                    )
                    zsb = mp.tile([128, NC, cnt * K], F32, tag="zsb", bufs=1,
                                  name=f"zsb{g}")
                    nc.gpsimd.dma_start(
                        zsb[:], bo[:].rearrange("(r p) f -> p r f", p=128))
                    z = mp.tile([128, cnt * K], F32, tag="z", bufs=2,
                                name=f"z{g}")
                    nc.vector.tensor_add(z[:], zsb[:, 0, :], zsb[:, 1, :])
                    for r in range(2, NC):
                        nc.vector.tensor_add(z[:], z[:], zsb[:, r, :])
                    nc.vector.tensor_scalar_add(z[:], z[:],
                                                -float((VP - VSH) * NC))
                    a = mp.tile([128, cnt, K], F32, tag="a", bufs=2,
                                name=f"a{g}")
                    nc.vector.reciprocal(a[:].rearrange("p t k -> p (t k)"),
                                         z[:])
                    nc.vector.tensor_mul(a[:], a[:], gw[:, ts : ts + cnt, :])
                    return a

                def emit_group_mix_dve(g, ts, cnt, pts, a):
                    for t2 in range(cnt):
                        t = ts + t2
                        # accumulate in place into the (dying) P tile's
                        # first-expert slice
                        m = pts[t2][:, 0:VSH]
                        nc.vector.tensor_scalar_mul(m, m, a[:, t2, 0:1])
                        for k in range(1, K):
                            mk = mp.tile([128, VSH], BF16, tag="mk", bufs=1,
                                         name=f"mk{g}_{t2}_{k}")
                            nc.vector.tensor_scalar_mul(
                                mk[:], pts[t2][:, k * VP : k * VP + VSH],
                                a[:, t2, k : k + 1])
                            nc.vector.tensor_add(m, m, mk[:])
                        ob = mp.tile([128, VSH], BF16, tag="mk", bufs=1,
                                     name=f"ob{g}_{t2}")
                        nc.scalar.activation(ob[:], m, AFT.Ln,
                                             bias=eps_log[:], scale=1.0)
                        nc.scalar.dma_start(o_d[t], ob[:])

                def emit_group_mix_pe(g, ts, cnt, pts, a):
                    """Mix on the (tail-idle) PE: psum[t,v] += diag(a_k)@P_k,
                    Ln straight from PSUM, chunked by 1024 cols."""
                    for t2 in range(cnt):
                        t = ts + t2
                        dgs = []
                        for k in range(K):
                            dk = mp.tile([128, 128], BF16, tag="diag", bufs=8,
                                         name=f"dk{g}_{t2}_{k}")
                            nc.vector.tensor_scalar_mul(dk[:], identB[:],
                                                        a[:, t2, k : k + 1])
                            dgs.append(dk)
                        for c4 in range(4):
                            width = 1024 if c4 < 3 else VSH - 3 * 1024
                            mt = ps.tile([128, 1024], F32, tag="mm", bufs=4,
                                         name=f"mx{g}_{t2}_{c4}")
                            for sub in range(2):
                                w2 = min(512, width - sub * 512)
                                if w2 <= 0:
                                    continue
                                for k in range(K):
                                    off = k * VP + c4 * 1024 + sub * 512
                                    nc.tensor.matmul(
                                        mt[:, sub * 512 : sub * 512 + w2],
                                        lhsT=dgs[k][:],
                                        rhs=pts[t2][:, off : off + w2],
                                        start=(k == 0), stop=(k == K - 1),
                                    )
                            ob = mp.tile([128, 1024], BF16, tag="obp", bufs=2,
                                         name=f"obp{g}_{t2}_{c4}")
                            nc.scalar.activation(ob[:, :width], mt[:, :width],
                                                 AFT.Ln, bias=eps_log[:],
                                                 scale=1.0)
                            nc.scalar.dma_start(
                                o_d[t, :, c4 * 1024 : c4 * 1024 + width],
                                ob[:, :width])

                # ---- emission ----
                # 1. group 0 / chunks 0-1 matmuls first (PE starts at ~+4us)
                g0_ts, g0_cnt = GROUPS[0]
                ssum0, pts0 = alloc_group(0, g0_cnt)
                pp0 = emit_gemm_chunk(0, g0_ts, g0_cnt, 0, pts0, ssum0,
                                      emit_exp=False)
                pp1 = emit_gemm_chunk(0, g0_ts, g0_cnt, 1, pts0, ssum0,
                                      emit_exp=False)

                # 1b. fast scl path: per-token sum(x^2) in token-partition
                # layout (DVE square + reduce, no PE/PSUM), then rsqrt via
                # exp(-0.5 ln) on two tiny [128,TB] activations.
                ss_tok = mp.tile([128, TB], F32, tag="sstok", bufs=1)
                with tc.tile_pool(name="fast", bufs=2) as fast_pool:
                    for tb in range(TB):
                        sqn = fast_pool.tile([128, H], BF16, tag="sqn", bufs=1)
                        nc.vector.tensor_mul(sqn[:], xn[:, tb, :],
                                             xn[:, tb, :])
                        nc.vector.tensor_reduce(ss_tok[:, tb : tb + 1],
                                                sqn[:],
                                                axis=mybir.AxisListType.X,
                                                op=mybir.AluOpType.add)
                lnt = mp.tile([128, TB], F32, tag="lnt", bufs=1)
                nc.scalar.activation(lnt[:], ss_tok[:], AFT.Ln,
                                     bias=eps_rms[:], scale=1.0 / H)
                nc.scalar.activation(scl[:], lnt[:], AFT.Exp,
                                     bias=0.0, scale=-0.5)
                nc.vector.tensor_scalar_mul(scl[:], scl[:], 1.0 / WSCALE)

                # 1c. backfill exps for the pre-norm chunks
                emit_exp_chunk(0, g0_ts, g0_cnt, 0, pts0, ssum0, pp0)
                emit_exp_chunk(0, g0_ts, g0_cnt, 1, pts0, ssum0, pp1)

                # 2. replicated-rinv path (gate only): ones-matmul column
                #    reduce; rsqrt via exp(-0.5 ln); in-place normalize of xT
                with tc.tile_pool(name="norm", bufs=2) as norm_pool:
                    rinvT = norm_pool.tile([128, T], BF16, tag="rinvT", bufs=1)
                    psst = ps.tile([128, 1024], F32, tag="mm", bufs=4,
                                   name="pss")
                    pss = [psst[:, :512], psst[:, 512:]]
                    for hb in range(HB):
                        for half in range(2):
                            sq = norm_pool.tile([128, 512], BF16, tag="sq", bufs=1)
                            xc = x8[:, hb, half * 512 : (half + 1) * 512]
                            nc.vector.tensor_mul(sq[:], xc, xc)
                            nc.tensor.matmul(
                                pss[half],
                                lhsT=ones[:],
                                rhs=sq[:],
                                start=(hb == 0), stop=(hb == HB - 1),
                            )
                    for half in range(2):
                        sl = slice(half * 512, (half + 1) * 512)
                        lnv = norm_pool.tile([128, 512], BF16, tag="sq", bufs=1)
                        nc.scalar.activation(lnv[:], pss[half], AFT.Ln,
                                             bias=eps_rms[:], scale=1.0 / H)
                        nc.scalar.activation(rinvT[:, sl], lnv[:], AFT.Exp,
                                             bias=0.0, scale=-0.5)
                    for tb in range(HB):
                        nc.vector.tensor_mul(xT[:, tb, :], xT[:, tb, :],
                                             rinvT[:])

                # 3. the rest of the groups; gate emitted mid-group-0
                state = {}
                for g, (ts, cnt) in enumerate(GROUPS):
                    if g == 0:
                        ssum, pts = ssum0, pts0
                        for gG in range(2, NG):
                            emit_gemm_chunk(g, ts, cnt, gG, pts, ssum)
                            if gG == 10:
                                emit_gate()
                    else:
                        ssum, pts = alloc_group(g, cnt)
                        for gG in range(NG):
                            emit_gemm_chunk(g, ts, cnt, gG, pts, ssum)
                    if g == 0:
                        pass
                    else:
                        ts_, cnt_, pts_, a_ = state[g - 1]
                        emit_group_mix_dve(g - 1, ts_, cnt_, pts_, a_)
                    a = emit_group_sync(g, ts, cnt, ssum)
                    state[g] = (ts, cnt, pts, a)
                gl = len(GROUPS) - 1
                emit_group_mix_pe(gl, state[gl][0], state[gl][1],
                                  state[gl][2], state[gl][3])
    nc.compile()
    return nc


_CACHE = {}


def _get_kernel():
    if "k" not in _CACHE:
        _CACHE["k"] = build_kernel()
    return _CACHE["k"]


def kernel(hidden_states, rms_scale, gate_down_w, gate_down_b, gate_up_w,
           gate_up_b, expert_w, trace=False):
    nc_k = _get_kernel()
    core_ids = list(range(NC))

    x = np.ascontiguousarray(
        np.asarray(hidden_states, dtype=np.float32).reshape(T, H))
    scale = np.asarray(rms_scale, dtype=np.float32)
    xT = np.ascontiguousarray(x.T)
    xT_bf = xT.astype(ml_dtypes.bfloat16)
    xT_f8 = xT.astype(FP8NP)
    xn_f8 = np.ascontiguousarray(x.reshape(TB, 128, H)).astype(FP8NP)
    # fold rms_scale into every weight that consumes the normed activations
    wd = (np.asarray(gate_down_w, dtype=np.float32)
          * scale[:, None]).astype(ml_dtypes.bfloat16)
    wu = np.asarray(gate_up_w, dtype=np.float32).astype(ml_dtypes.bfloat16)
    bd = np.ascontiguousarray(
        np.asarray(gate_down_b, dtype=np.float32).reshape(D, 1))
    bu = np.ascontiguousarray(np.tile(
        np.asarray(gate_up_b, dtype=np.float32).reshape(1, K), (128, 1)))
    we = np.asarray(expert_w, dtype=np.float32) * scale[:, None]

    in_maps = []
    for c in range(NC):
        wsh = np.zeros((H, C), dtype=FP8NP)
        for k in range(K):
            wsh[:, k * VP : k * VP + VSH] = (
                we[:, k * V + c * VSH : k * V + (c + 1) * VSH] * WSCALE
            ).astype(FP8NP)
        # block to [gG, hs, p, j, c]: row h = hs*256 + j*128 + p,
        # col = gG*1024 + c
        wr = np.ascontiguousarray(
            wsh.reshape(HB // 2, 2, 128, NG, 1024).transpose(3, 0, 2, 1, 4))
        in_maps.append({"xt": xT_bf, "x8": xT_f8, "xn": xn_f8, "w": wr,
                        "wd": wd, "wu": wu, "bd": bd, "nbd": -bd, "bu": bu})

    res = run_bass_kernel_spmd(nc_k, in_maps, core_ids, trace=trace)

    out = np.empty((T, V), dtype=np.float32)
    for c in range(NC):
        out[:, c * VSH : (c + 1) * VSH] = (
            res.results[c]["o"].astype(np.float32).reshape(T, VSH))
    out = out.reshape(B, S, V)
    if trace:
        return out, (res, res)
    return out


# revision 37
# speedup vs baseline: 1.0185x; 1.0185x over previous
"""Trainium2 Bass kernel for MixtureOfSoftmaxes (v3).

Module: RMSNorm -> gate MLP (silu, softmax over K experts) -> big GEMM
x @ expert_w (H=1024 -> K*V=128000), softmax over V per expert, mix with
gate weights, log.

Sharding: tensor-parallel over vocab. Core c computes, for all K=4
experts, the vocab window [c*4000, (c+1)*4000) (padded to 4096/expert).
Per token group (2 blocks of 128 tokens) each core AllGathers its local
softmax partial sums (4 KB) and reduces them on DVE; the mix then uses
a = softmax(gate) / Z.

v3 structure:
 - Host pre-transposes x to [H, T] (bf16 + fp8) and pre-blocks W into
   contiguous 256 KB (col-chunk, hs) tiles for full-rate DMA.
 - RMS rinv via ones-matmul column reduction + exp(-0.5*ln(v)); squares
   taken from the fp8 copy (arrives first). rinv folds into the big
   GEMM's exp as its per-partition scale; the gate path normalizes xT
   in place.
 - Big GEMM: 4 token groups x 16 col-chunks of 1024; psum tiles
   [128,1024]; wide exp+row-sum ACTIVATE per (t-block, chunk); P kept
   in SBUF bf16. First col-chunk's matmuls are emitted ahead of the
   norm matmuls so the PE starts at ~+4us.
 - Collective bounce DMAs ride the GpSimd queue so the Sync queue's W
   stream never blocks on an AllGather.
 - Groups 0-2 mix on DVE (hidden under the next group's GEMM); the
   last group mixes on the then-idle PE via accumulating diag(a_k)@P_k
   matmuls, with Ln reading straight from PSUM.
"""

import sys

sys.path.insert(0, "/opt/trn_rl_repo")

import numpy as np
import ml_dtypes

import concourse.bass as bass
import concourse.bacc as bacc
import concourse.mybir as mybir
import concourse.tile as tile
from concourse.bass_utils import run_bass_kernel_spmd
from concourse.masks import make_identity

AFT = mybir.ActivationFunctionType
F32 = mybir.dt.float32
BF16 = mybir.dt.bfloat16
FP8 = mybir.dt.float8e4
FP8NP = ml_dtypes.float8_e4m3
WSCALE = 16.0

B, S, H, K, V = 2, 512, 1024, 4, 32000
T = B * S              # 1024 tokens
NC = 8                 # cores
VSH = V // NC          # 4000 vocab cols per core per expert
VP = 4096              # padded per-expert width
C = K * VP             # 16384 GEMM cols per core
D = H // 2             # 512 gate hidden
EPS_RMS = 1e-5
EPS_LOG = 1e-10
TB = T // 128          # 8 token blocks
HB = H // 128          # 8 contraction blocks
NG = C // 1024         # 16 col chunks of 1024
GROUPS = [(0, 2), (2, 2), (4, 2), (6, 2)]  # (start t-block, count)


_ONE_SET = "natural_log_exp_and_others"
_orig_gat = None


def _single_set_tables(arch):
    """Keep id->set alignment but strip our functions from every set except
    the combined exp+ln one, so the table-load pass never alternates sets."""
    d = _orig_gat(arch)
    if _ONE_SET not in d:
        return d
    combined = d[_ONE_SET]
    return {name: (fns if name == _ONE_SET else fns - combined)
            for name, fns in d.items()}


def build_kernel():
    global _orig_gat
    if _orig_gat is None:
        _orig_gat = bacc.get_activation_tables
        bacc.get_activation_tables = _single_set_tables
    nc = bacc.Bacc("TRN2", target_bir_lowering=False, debug=False, num_devices=NC)
    xT_d = nc.dram_tensor("xt", [H, T], BF16, kind="ExternalInput")
    xn_d = nc.dram_tensor("xn", [TB, 128, H], FP8, kind="ExternalInput")
    x8_d = nc.dram_tensor("x8", [H, T], FP8, kind="ExternalInput")
    # W pre-blocked on host: [gG, hs, p, j, c] so each (gG, hs) chunk is
    # one contiguous 256 KB read.
    w_d = nc.dram_tensor("w", [NG, HB // 2, 128, 2, 1024], FP8,
                         kind="ExternalInput")
    wd_d = nc.dram_tensor("wd", [H, D], BF16, kind="ExternalInput")
    wu_d = nc.dram_tensor("wu", [D, K], BF16, kind="ExternalInput")
    bd_d = nc.dram_tensor("bd", [D, 1], F32, kind="ExternalInput")
    nbd_d = nc.dram_tensor("nbd", [D, 1], F32, kind="ExternalInput")
    bu_d = nc.dram_tensor("bu", [128, K], F32, kind="ExternalInput")
    o_d = nc.dram_tensor("o", [TB, 128, VSH], BF16, kind="ExternalOutput")

    xT_ap = xT_d.rearrange("(hb p) t -> p hb t", p=128)
    x8_ap = x8_d.rearrange("(hb p) t -> p hb t", p=128)
    wd_ap = wd_d.rearrange("(hb p) d -> p hb d", p=128)
    wu_ap = wu_d.rearrange("(db p) k -> p db k", p=128)
    bd_ap = bd_d.rearrange("(db p) o -> p db o", p=128)
    nbd_ap = nbd_d.rearrange("(db p) o -> p db o", p=128)

    with tile.TileContext(nc) as tc:
        with tc.tile_pool(name="persist", bufs=1) as pers, \
             tc.tile_pool(name="ps", bufs=1, space="PSUM") as ps, \
             tc.tile_pool(name="ccdr", bufs=1, space="DRAM") as ccdr:
            # ---- persistent small tiles ----
            identB = pers.tile([128, 128], BF16)
            make_identity(nc, identB[:])
            ones = pers.tile([128, 128], BF16)
            nc.gpsimd.memset(ones[:], 1.0)
            eps_rms = pers.tile([128, 1], F32)
            nc.gpsimd.memset(eps_rms[:], EPS_RMS)
            eps_log = pers.tile([128, 1], F32)
            nc.gpsimd.memset(eps_log[:], EPS_LOG)
            xT = pers.tile([128, HB, T], BF16)    # 16 KB/part (normalized in place)
            x8 = pers.tile([128, HB, T], FP8)     # 8 KB/part (raw x, fp8)
            scl = pers.tile([128, TB], F32)       # rinv/WSCALE per t-block
            gw = pers.tile([128, TB, K], F32)     # gate softmax weights
            wd_sb = pers.tile([128, HB, D], BF16)   # 8 KB/part
            wu_sb = pers.tile([128, D // 128, K], BF16)
            bd_sb = pers.tile([128, D // 128, 1], F32)
            nbd_sb = pers.tile([128, D // 128, 1], F32)
            buT_sb = pers.tile([128, K], F32)
            gT = pers.tile([128, D // 128, T], BF16)  # 8 KB/part

            # ---- input DMAs (x8 chunked first: feeds squares + GEMM) ----
            for hb in range(HB):
                nc.sync.dma_start(x8[:, hb, :], x8_ap[:, hb, :])
            xn = pers.tile([128, TB, H], FP8)     # 8 KB/part (fast scl path)
            # scalar-queue (qActDynamicHW) so xn lands in parallel with x8
            for tb in range(TB):
                nc.scalar.dma_start(xn[:, tb, :], xn_d[tb])
            for hb in range(HB):
                nc.sync.dma_start(xT[:, hb, :], xT_ap[:, hb, :])
            nc.sync.dma_start(wd_sb[:], wd_ap)
            nc.sync.dma_start(wu_sb[:], wu_ap)
            nc.sync.dma_start(bd_sb[:], bd_ap)
            nc.sync.dma_start(nbd_sb[:], nbd_ap)
            nc.sync.dma_start(buT_sb[:], bu_d[:])
            # warmup AllGather: pays the collective first-call staging cost
            # off the critical path (overlaps the norm + first GEMM chunks)
            wup = pers.tile([128, 8], F32)
            nc.gpsimd.memset(wup[:], 0.0)
            bi_w = ccdr.tile([128, 8], F32, tag="bi", bufs=2, name="bi_w")
            bo_w = ccdr.tile([NC * 128, 8], F32, tag="bo", bufs=2, name="bo_w")
            nc.gpsimd.dma_start(bi_w[:], wup[:])
            nc.gpsimd.collective_compute(
                "AllGather", mybir.AluOpType.bypass,
                replica_groups=[list(range(NC))],
                ins=[bi_w[:]], outs=[bo_w[:]],
            )

            with tc.tile_pool(name="main", bufs=1) as mp:

                def emit_exp_chunk(g, ts, cnt, gG, pts, ssum, pp):
                    for t2 in range(cnt):
                        t = ts + t2
                        kk, cc = gG // (NG // K), gG % (NG // K)
                        nc.scalar.activation(
                            pts[t2][:, gG * 1024 : (gG + 1) * 1024],
                            pp[t2][:], AFT.Exp,
                            bias=0.0, scale=scl[:, t : t + 1],
                            accum_out=ssum[:, t2, kk, cc : cc + 1],
                        )

                def emit_gemm_chunk(g, ts, cnt, gG, pts, ssum, emit_exp=True):
                    """One 1024-col chunk of a token group's GEMM."""
                    pp = []
                    for t2 in range(cnt):
                        pp.append(ps.tile([128, 1024], F32, tag="mm", bufs=4,
                                          name=f"mm{g}_{gG}_{t2}"))
                    for hs in range(HB // 2):
                        wt = mp.tile([128, 2, 1024], FP8, tag="wt", bufs=6,
                                     name=f"wt{g}_{gG}_{hs}")
                        nc.sync.dma_start(wt[:], w_d[gG, hs])
                        for t2 in range(cnt):
                            t = ts + t2
                            for ch in range(2):
                                nc.tensor.matmul(
                                    pp[t2][:, ch * 512 : (ch + 1) * 512],
                                    lhsT=x8[:, 2 * hs : 2 * hs + 2,
                                            t * 128 : (t + 1) * 128],
                                    rhs=wt[:, :, ch * 512 : (ch + 1) * 512],
                                    start=(hs == 0), stop=(hs == HB // 2 - 1),
                                    perf_mode=mybir.MatmulPerfMode.DoubleRow,
                                )
                    if emit_exp:
                        emit_exp_chunk(g, ts, cnt, gG, pts, ssum, pp)
                    return pp

                def alloc_group(g, cnt):
                    ssum = mp.tile([128, cnt, K, NG // K], F32, tag="ssum",
                                   bufs=2, name=f"ssum{g}")
                    pts = [mp.tile([128, C], BF16, tag="P", bufs=4,
                                   name=f"P{g}_{t2}") for t2 in range(cnt)]
                    return ssum, pts

                def emit_gate():
                    for d in range(D // 128):
                        for half in range(2):
                            sl = slice(half * 512, (half + 1) * 512)
                            pgt = ps.tile([128, 1024], F32, tag="mm",
                                          name=f"pg{d}_{half}", bufs=4)
                            pg = pgt[:, :512]
                            for hb in range(HB):
                                nc.tensor.matmul(
                                    pg,
                                    lhsT=wd_sb[:, hb, d * 128 : (d + 1) * 128],
                                    rhs=xT[:, hb, sl],
                                    start=(hb == 0), stop=(hb == HB - 1),
                                )
                            lin = mp.tile([128, 512], BF16, tag="glin", bufs=1,
                                          name=f"lin{d}_{half}")
                            nc.scalar.activation(lin[:], pg, AFT.Identity,
                                                 bias=bd_sb[:, d, :], scale=1.0)
                            ex = pgt[:, 512:]
                            nc.scalar.activation(ex, pg, AFT.Exp,
                                                 bias=nbd_sb[:, d, :], scale=-1.0)
                            nc.vector.tensor_scalar_add(ex, ex, 1.0)
                            nc.vector.reciprocal_approx_fast(ex, ex)
                            nc.vector.tensor_mul(gT[:, d, sl], lin[:], ex)
                    glt = mp.tile([128, TB, K], F32, tag="glt", bufs=1)
                    for tb in range(TB):
                        pgl = ps.tile([128, 1024], F32, tag="mm",
                                      name=f"pgl{tb}", bufs=4)
                        for d in range(D // 128):
                            nc.tensor.matmul(
                                pgl[:, :4],
                                lhsT=gT[:, d, tb * 128 : (tb + 1) * 128],
                                rhs=wu_sb[:, d, :],
                                start=(d == 0), stop=(d == D // 128 - 1),
                            )
                        nc.vector.tensor_add(glt[:, tb, :], pgl[:, :4],
                                             buT_sb[:])
                    negm = mp.tile([128, TB], F32, tag="negm", bufs=1)
                    esum = mp.tile([128, TB], F32, tag="esum", bufs=1)
                    for tb in range(TB):
                        nc.vector.tensor_reduce(
                            negm[:, tb : tb + 1], glt[:, tb, :],
                            axis=mybir.AxisListType.X, op=mybir.AluOpType.max,
                            negate=True,
                        )
                        nc.scalar.activation(gw[:, tb, :], glt[:, tb, :],
                                             AFT.Exp,
                                             bias=negm[:, tb : tb + 1],
                                             scale=1.0,
                                             accum_out=esum[:, tb : tb + 1])
                    rsum = mp.tile([128, TB], F32, tag="rsum", bufs=1)
                    nc.vector.reciprocal(rsum[:], esum[:])
                    for tb in range(TB):
                        nc.vector.tensor_scalar_mul(gw[:, tb, :], gw[:, tb, :],
                                                    rsum[:, tb : tb + 1])

                def emit_group_sync(g, ts, cnt, ssum):
                    s_g = mp.tile([128, cnt, K], F32, tag="s_g", bufs=2,
                                  name=f"s_g{g}")
                    nc.vector.tensor_reduce(
                        s_g[:], ssum[:],
                        axis=mybir.AxisListType.X, op=mybir.AluOpType.add,
                    )
                    bi = ccdr.tile([128, cnt * K], F32, tag="bi", bufs=2,
                                   name=f"bi{g}")
                    bo = ccdr.tile([NC * 128, cnt * K], F32, tag="bo", bufs=2,
                                   name=f"bo{g}")
                    nc.gpsimd.dma_start(bi[:],
                                        s_g[:].rearrange("p t k -> p (t k)"))
                    nc.gpsimd.collective_compute(
                        "AllGather", mybir.AluOpType.bypass,
                        replica_groups=[list(range(NC))],
                        ins=[bi[:]], outs=[bo[:]],
                    )
                    zsb = mp.tile([128, NC, cnt * K], F32, tag="zsb", bufs=1,
                                  name=f"zsb{g}")
                    nc.gpsimd.dma_start(
                        zsb[:], bo[:].rearrange("(r p) f -> p r f", p=128))
                    z = mp.tile([128, cnt * K], F32, tag="z", bufs=2,
                                name=f"z{g}")
                    nc.vector.tensor_add(z[:], zsb[:, 0, :], zsb[:, 1, :])
                    for r in range(2, NC):
                        nc.vector.tensor_add(z[:], z[:], zsb[:, r, :])
                    nc.vector.tensor_scalar_add(z[:], z[:],
                                                -float((VP - VSH) * NC))
                    a = mp.tile([128, cnt, K], F32, tag="a", bufs=2,
                                name=f"a{g}")
                    nc.vector.reciprocal(a[:].rearrange("p t k -> p (t k)"),
                                         z[:])
                    nc.vector.tensor_mul(a[:], a[:], gw[:, ts : ts + cnt, :])
                    return a

                def emit_group_mix_dve(g, ts, cnt, pts, a):
                    for t2 in range(cnt):
                        t = ts + t2
                        # accumulate in place into the (dying) P tile's
                        # first-expert slice
                        m = pts[t2][:, 0:VSH]
                        nc.vector.tensor_scalar_mul(m, m, a[:, t2, 0:1])
                        for k in range(1, K):
                            mk = mp.tile([128, VSH], BF16, tag="mk", bufs=1,
                                         name=f"mk{g}_{t2}_{k}")
                            nc.vector.tensor_scalar_mul(
                                mk[:], pts[t2][:, k * VP : k * VP + VSH],
                                a[:, t2, k : k + 1])
                            nc.vector.tensor_add(m, m, mk[:])
                        ob = mp.tile([128, VSH], BF16, tag="mk", bufs=1,
                                     name=f"ob{g}_{t2}")
                        nc.scalar.activation(ob[:], m, AFT.Ln,
                                             bias=eps_log[:], scale=1.0)
                        nc.scalar.dma_start(o_d[t], ob[:])

                def emit_group_mix_pe(g, ts, cnt, pts, a):
                    """Mix on the (tail-idle) PE: psum[t,v] += diag(a_k)@P_k,
                    Ln straight from PSUM, chunked by 1024 cols."""
                    for t2 in range(cnt):
                        t = ts + t2
                        dgs = []
                        for k in range(K):
                            dk = mp.tile([128, 128], BF16, tag="diag", bufs=8,
                                         name=f"dk{g}_{t2}_{k}")
                            nc.vector.tensor_scalar_mul(dk[:], identB[:],
                                                        a[:, t2, k : k + 1])
                            dgs.append(dk)
                        for c4 in range(4):
                            width = 1024 if c4 < 3 else VSH - 3 * 1024
                            mt = ps.tile([128, 1024], F32, tag="mm", bufs=4,
                                         name=f"mx{g}_{t2}_{c4}")
                            for sub in range(2):
                                w2 = min(512, width - sub * 512)
                                if w2 <= 0:
                                    continue
                                for k in range(K):
                                    off = k * VP + c4 * 1024 + sub * 512
                                    nc.tensor.matmul(
                                        mt[:, sub * 512 : sub * 512 + w2],
                                        lhsT=dgs[k][:],
                                        rhs=pts[t2][:, off : off + w2],
                                        start=(k == 0), stop=(k == K - 1),
                                    )
                            ob = mp.tile([128, 1024], BF16, tag="obp", bufs=2,
                                         name=f"obp{g}_{t2}_{c4}")
                            nc.scalar.activation(ob[:, :width], mt[:, :width],
                                                 AFT.Ln, bias=eps_log[:],
                                                 scale=1.0)
                            nc.scalar.dma_start(
                                o_d[t, :, c4 * 1024 : c4 * 1024 + width],
                                ob[:, :width])

                # ---- emission ----
                # 1. group 0 / chunks 0-1 matmuls first (PE starts at ~+4us)
                g0_ts, g0_cnt = GROUPS[0]
                ssum0, pts0 = alloc_group(0, g0_cnt)
                pp0 = emit_gemm_chunk(0, g0_ts, g0_cnt, 0, pts0, ssum0,
                                      emit_exp=False)
                pp1 = emit_gemm_chunk(0, g0_ts, g0_cnt, 1, pts0, ssum0,
                                      emit_exp=False)

                # 1b. fast scl path: per-token sum(x^2) in token-partition
                # layout (DVE square + reduce, no PE/PSUM), then rsqrt via
                # exp(-0.5 ln) on two tiny [128,TB] activations.
                ss_tok = mp.tile([128, TB], F32, tag="sstok", bufs=1)
                with tc.tile_pool(name="fast", bufs=2) as fast_pool:
                    for tb in range(TB):
                        sqn = fast_pool.tile([128, H], BF16, tag="sqn", bufs=1)
                        nc.vector.tensor_mul(sqn[:], xn[:, tb, :],
                                             xn[:, tb, :])
                        nc.vector.tensor_reduce(ss_tok[:, tb : tb + 1],
                                                sqn[:],
                                                axis=mybir.AxisListType.X,
                                                op=mybir.AluOpType.add)
                lnt = mp.tile([128, TB], F32, tag="lnt", bufs=1)
                nc.scalar.activation(lnt[:], ss_tok[:], AFT.Ln,
                                     bias=eps_rms[:], scale=1.0 / H)
                nc.scalar.activation(scl[:], lnt[:], AFT.Exp,
                                     bias=0.0, scale=-0.5)
                nc.vector.tensor_scalar_mul(scl[:], scl[:], 1.0 / WSCALE)

                # 1c. backfill exps for the pre-norm chunks
                emit_exp_chunk(0, g0_ts, g0_cnt, 0, pts0, ssum0, pp0)
                emit_exp_chunk(0, g0_ts, g0_cnt, 1, pts0, ssum0, pp1)

                # 2. replicated-rinv path (gate only): ones-matmul column
                #    reduce; rsqrt via exp(-0.5 ln); in-place normalize of xT
                with tc.tile_pool(name="norm", bufs=2) as norm_pool:
                    rinvT = norm_pool.tile([128, T], BF16, tag="rinvT", bufs=1)
                    psst = ps.tile([128, 1024], F32, tag="mm", bufs=4,
                                   name="pss")
                    pss = [psst[:, :512], psst[:, 512:]]
                    for hb in range(HB):
                        for half in range(2):
                            sq = norm_pool.tile([128, 512], BF16, tag="sq", bufs=1)
                            xc = x8[:, hb, half * 512 : (half + 1) * 512]
                            nc.vector.tensor_mul(sq[:], xc, xc)
                            nc.tensor.matmul(
                                pss[half],
                                lhsT=ones[:],
                                rhs=sq[:],
                                start=(hb == 0), stop=(hb == HB - 1),
                            )
                    for half in range(2):
                        sl = slice(half * 512, (half + 1) * 512)
                        lnv = norm_pool.tile([128, 512], BF16, tag="sq", bufs=1)
                        nc.scalar.activation(lnv[:], pss[half], AFT.Ln,
                                             bias=eps_rms[:], scale=1.0 / H)
                        nc.scalar.activation(rinvT[:, sl], lnv[:], AFT.Exp,
                                             bias=0.0, scale=-0.5)
                    for tb in range(HB):
                        nc.vector.tensor_mul(xT[:, tb, :], xT[:, tb, :],
                                             rinvT[:])

                # 3. the rest of the groups; gate emitted mid-group-0
                state = {}
                for g, (ts, cnt) in enumerate(GROUPS):
                    if g == 0:
                        ssum, pts = ssum0, pts0
                        for gG in range(2, NG):
                            emit_gemm_chunk(g, ts, cnt, gG, pts, ssum)
                            if gG == 10:
                                emit_gate()
                    else:
                        ssum, pts = alloc_group(g, cnt)
                        for gG in range(NG):
                            emit_gemm_chunk(g, ts, cnt, gG, pts, ssum)
                    if g == 0:
                        pass
                    else:
                        ts_, cnt_, pts_, a_ = state[g - 1]
                        emit_group_mix_dve(g - 1, ts_, cnt_, pts_, a_)
                    a = emit_group_sync(g, ts, cnt, ssum)
                    state[g] = (ts, cnt, pts, a)
                gl = len(GROUPS) - 1
                emit_group_mix_pe(gl, state[gl][0], state[gl][1],
                                  state[gl][2], state[gl][3])
    nc.compile()
    return nc


_CACHE = {}


def _get_kernel():
    if "k" not in _CACHE:
        _CACHE["k"] = build_kernel()
    return _CACHE["k"]


def kernel(hidden_states, rms_scale, gate_down_w, gate_down_b, gate_up_w,
           gate_up_b, expert_w, trace=False):
    nc_k = _get_kernel()
    core_ids = list(range(NC))

    x = np.ascontiguousarray(
        np.asarray(hidden_states, dtype=np.float32).reshape(T, H))
    scale = np.asarray(rms_scale, dtype=np.float32)
    xT = np.ascontiguousarray(x.T)
    xT_bf = xT.astype(ml_dtypes.bfloat16)
    xT_f8 = xT.astype(FP8NP)
    xn_f8 = np.ascontiguousarray(x.reshape(TB, 128, H)).astype(FP8NP)
    # fold rms_scale into every weight that consumes the normed activations
    wd = (np.asarray(gate_down_w, dtype=np.float32)
          * scale[:, None]).astype(ml_dtypes.bfloat16)
    wu = np.asarray(gate_up_w, dtype=np.float32).astype(ml_dtypes.bfloat16)
    bd = np.ascontiguousarray(
        np.asarray(gate_down_b, dtype=np.float32).reshape(D, 1))
    bu = np.ascontiguousarray(np.tile(
        np.asarray(gate_up_b, dtype=np.float32).reshape(1, K), (128, 1)))
    we = np.asarray(expert_w, dtype=np.float32) * scale[:, None]

    in_maps = []
    for c in range(NC):
        wsh = np.zeros((H, C), dtype=FP8NP)
        for k in range(K):
            wsh[:, k * VP : k * VP + VSH] = (
                we[:, k * V + c * VSH : k * V + (c + 1) * VSH] * WSCALE
            ).astype(FP8NP)
        # block to [gG, hs, p, j, c]: row h = hs*256 + j*128 + p,
        # col = gG*1024 + c
        wr = np.ascontiguousarray(
            wsh.reshape(HB // 2, 2, 128, NG, 1024).transpose(3, 0, 2, 1, 4))
        in_maps.append({"xt": xT_bf, "x8": xT_f8, "xn": xn_f8, "w": wr,
                        "wd": wd, "wu": wu, "bd": bd, "nbd": -bd, "bu": bu})

    res = run_bass_kernel_spmd(nc_k, in_maps, core_ids, trace=trace)

    out = np.empty((T, V), dtype=np.float32)
    for c in range(NC):
        out[:, c * VSH : (c + 1) * VSH] = (
            res.results[c]["o"].astype(np.float32).reshape(T, VSH))
    out = out.reshape(B, S, V)
    if trace:
        return out, (res, res)
    return out
